# revision 33
# baseline (speedup 1.0000x reference)
# Trainium2 Bass kernel for LinearAttention (nn_LinearAttention_87686052315975).
#
# Reference computation (per batch element b of 16):
#   xf = x[b].reshape(512, 4096)                      # [c, l]
#   qkv = w_qkv @ xf; q, k, v split into 8 heads x 64 dims
#   k = softmax(k, axis=l)
#   context_h = k_h @ v_h^T                           # [64, 64]
#   out_h = context_h^T @ q_h                          # [64, l]
#   y = w_out @ concat(out_h) + b_out                 # [512, l]
#
# Key restructure vs a direct mapping: since context_h is tiny, fold it into
# the weights.  y = sum_h Wout_h ctxn_h^T Wq_h x = M x with M [512, 512]
# depending only on ctx (data-dependent) and the fixed weights.  This removes
# the q projection (q never materialized) and the per-l attention pass;
# after the k/v sweep we build M (~10k PE cycles) and do one plain matmul
# y = M x + bias.
#
# Per-batch structure (2 batches per core, data-parallel over 8 cores):
#   Pass 1 (l chunked by 512):  kT/vT computed transposed (l on partitions)
#     so the context contraction over l maps onto the PE K dim; E = exp(kT)
#     cast to bf16; vT cast to bf16 with a ones column per head appended so
#     the context matmul also accumulates rowsum(E) (softmax denominator).
#     ctx accumulates in PSUM across all 32 l-subtiles (2 head-pairs per
#     bank, block-diagonal packing).  bf16 runs the N=132 context matmuls at
#     1 cycle/row (fp32r would pay 4x at N<256).  x is also cast to a
#     resident bf16 copy for pass 2.
#   Finalize:  ctxn = ctx * (1/rowsum) into block-diag bf16 tiles.
#   Build M:   A_pair = ctxn_pair^T-contract-Wq_pair  [128, 512]
#              M^T[c, o] = sum_pairs A_pair^T-contract-WoutT_pair
#   Pass 2:    y = (M^T)^T-contract-x_bf16 + bias; DMA out.
#
# Big fp32 matmuls (k/v projection) run as float32r (1 cycle/row at N>=512).
# Everything downstream of exp runs bf16 (inputs only; PSUM accumulation is
# fp32) — well inside the 2e-2 tolerance.

import numpy as np
from contextlib import ExitStack

import concourse.bass as bass
import concourse.bacc as bacc
import concourse.mybir as mybir
import concourse.tile as tile

# ---- problem constants (hardcoded per contract) ----
B, DIM, HGT, WID = 16, 512, 64, 64
L = HGT * WID            # 4096
HEADS, DH = 8, 64
HIDDEN = HEADS * DH      # 512
NCORES = 8
BPC = B // NCORES        # 2 batches per core
P = 128
CHUNK = 512
NCHUNK = L // CHUNK      # 8
KT = DIM // P            # 4 contraction tiles over channels
MT = DIM // P            # 4 output row tiles
LM = CHUNK // P          # 4 l-subtiles per chunk
NPAIR = HEADS // 2       # 4 head pairs
VW = DH + 2              # per-head vT width: 64 v cols + 2 ones cols (even N)
CTXW = 2 * VW            # 132: one pair's context block width

F32 = mybir.dt.float32
F32R = mybir.dt.float32r
BF16 = mybir.dt.bfloat16


def _f32(ap):
    return ap.bitcast(F32)


def build_kernel(ctx: ExitStack, tc: "tile.TileContext", x_in, wkvT_in, wq_in,
                 woutT_in, bias_in, y_out):
    nc = tc.nc

    wpool = ctx.enter_context(tc.tile_pool(name="weights", bufs=1))
    xpool = ctx.enter_context(tc.tile_pool(name="xc", bufs=8))
    xbpool = ctx.enter_context(tc.tile_pool(name="xbf", bufs=8))
    epool = ctx.enter_context(tc.tile_pool(name="ev", bufs=8))
    cpool = ctx.enter_context(tc.tile_pool(name="ctxp", bufs=8))
    apool = ctx.enter_context(tc.tile_pool(name="absf", bufs=4))
    mpool = ctx.enter_context(tc.tile_pool(name="mtbf", bufs=8))
    rpool = ctx.enter_context(tc.tile_pool(name="recip", bufs=8))
    ypool = ctx.enter_context(tc.tile_pool(name="ysb", bufs=5))
    psmm = ctx.enter_context(tc.tile_pool(name="psmm", bufs=4, space="PSUM"))
    # "ctx" needs only 2 bufs (finalize(b) drains before pass1(b+1)
    # allocates); the freed banks serve as extra pass-2 accumulators so the
    # tensor engine isn't gated on the bias-add drain at chunk boundaries.
    psctx = ctx.enter_context(tc.tile_pool(name="psctx", bufs=2, space="PSUM"))

    # ---- load k/v weights + bias up front; wq/wout deferred (only needed
    # by build-M, which runs after both k/v sweeps) so the first x chunk's
    # DMAs aren't queued behind weight traffic.
    wkv_sb = [wpool.tile([P, 2 * HIDDEN], F32R, tag=f"wkv{k}", name=f"wkv{k}")
              for k in range(KT)]
    bias_sb = wpool.tile([P, MT], F32, tag="bias", name="bias")
    nc.sync.dma_start(bias_sb[:], bias_in[:])
    wq_bf = wpool.tile([P, KT * DIM], BF16, tag="wq", name="wq")
    wout_bf = wpool.tile([P, KT * DIM], BF16, tag="wout", name="wout")

    def load_late_weights():
        # staged fp32 loads + on-chip bf16 casts (gpsimd SWDGE could cast in
        # flight, but its teardown costs ~12us of epilogue on the trace)
        sq = xpool.tile([P, KT * DIM], F32, tag="stg", name="wq_stage",
                        bufs=2)
        nc.sync.dma_start(
            sq[:].rearrange("p (k c) -> p k c", k=KT),
            wq_in[:, :].rearrange("(k p) c -> p k c", p=P))
        nc.vector.tensor_copy(wq_bf[:], sq[:])
        so = xpool.tile([P, KT * DIM], F32, tag="stg", name="wout_stage",
                        bufs=2)
        nc.sync.dma_start(
            so[:].rearrange("p (k c) -> p k c", k=KT),
            woutT_in[:, :].rearrange("(k p) c -> p k c", p=P))
        nc.scalar.copy(wout_bf[:], so[:])

    x_bf = {}      # batch -> 4 resident bf16 tiles [128, 4096]
    ctxP = {}      # batch -> 4 block-diag bf16 [128, 128] normalized ctx
    ctx_ps = {}    # batch -> 2 PSUM tiles [128, 264] (2 pairs each)

    def pass1(b):
        x_bf[b] = [xbpool.tile([P, L], BF16, tag="xbf", name=f"xbf{b}_{k}")
                   for k in range(KT)]
        ctx_ps[b] = [psctx.tile([P, 2 * CTXW], F32, tag="ctx", name="ctx")
                     for _ in range(2)]
        for i in range(NCHUNK):
            ls = slice(i * CHUNK, (i + 1) * CHUNK)
            # whole chunk in ONE dma (descriptors still fan out across all
            # DMA engines; fewer sync-engine issue slots).  The very first
            # chunk is split per k-tile and interleaved with the wkv loads
            # so the first matmul only waits on ~768 KB, not 3 MB.
            xcw = xpool.tile([P, KT * CHUNK], F32R, tag="xc", name="xcw",
                             bufs=4)
            if b == 0 and i == 0:
                # fine-grained first loads, k-projection weights first, so
                # the very first matmul only waits on ~384 KB of the ramp-up
                # limited early DMA window
                for k in range(KT):
                    nc.sync.dma_start(
                        xcw[:, k * CHUNK:k * CHUNK + CHUNK // 2],
                        x_in[b, k * P:(k + 1) * P, i * CHUNK:
                             i * CHUNK + CHUNK // 2])
                    nc.sync.dma_start(wkv_sb[k][:, 0:HIDDEN],
                                      wkvT_in[k * P:(k + 1) * P, 0:HIDDEN])
                    nc.sync.dma_start(wkv_sb[k][:, HIDDEN:2 * HIDDEN],
                                      wkvT_in[k * P:(k + 1) * P,
                                              HIDDEN:2 * HIDDEN])
                for k in range(KT):
                    nc.sync.dma_start(
                        xcw[:, k * CHUNK + CHUNK // 2:(k + 1) * CHUNK],
                        x_in[b, k * P:(k + 1) * P, i * CHUNK + CHUNK // 2:
                             (i + 1) * CHUNK])
            else:
                nc.sync.dma_start(
                    xcw[:].rearrange("p (k l) -> p k l", k=KT),
                    x_in[b, :, ls].rearrange("(k p) l -> p k l", p=P))
            xc = [xcw[:, k * CHUNK:(k + 1) * CHUNK] for k in range(KT)]
            for k in range(KT):
                nc.scalar.copy(x_bf[b][k][:, ls], _f32(xc[k]))

            E_t, vT_t = [], []
            for lm in range(LM):
                lms = slice(lm * P, (lm + 1) * P)
                # kT: [128 l, 512 (h,d)] -> E = exp
                ps = psmm.tile([P, CHUNK], F32, tag="mm", name="mm")
                for k in range(KT):
                    nc.tensor.matmul(ps[:], xc[k][:, lms],
                                     wkv_sb[k][:, 0:HIDDEN],
                                     start=(k == 0), stop=(k == KT - 1))
                e = epool.tile([P, CHUNK], BF16, tag="E", name="E")
                nc.scalar.activation(e[:], ps[:],
                                     mybir.ActivationFunctionType.Exp)
                E_t.append(e)
                # vT: [128 l, 512 (h,e)] -> bf16 with ones cols per head
                ps = psmm.tile([P, CHUNK], F32, tag="mm", name="mm")
                for k in range(KT):
                    nc.tensor.matmul(ps[:], xc[k][:, lms],
                                     wkv_sb[k][:, HIDDEN:2 * HIDDEN],
                                     start=(k == 0), stop=(k == KT - 1))
                v = epool.tile([P, HEADS * VW], BF16, tag="vT", name="vT")
                v_view = v[:].rearrange("p (h e) -> p h e", e=VW)
                nc.vector.tensor_copy(
                    v_view[:, :, 0:DH],
                    ps[:].rearrange("p (h e) -> p h e", e=DH))
                nc.vector.memset(v_view[:, :, DH:VW], 1.0)
                vT_t.append(v)

            # context accumulation into persistent PSUM, one matmul per
            # head pair (block-diag packing; off-diag blocks never read).
            # start=True resets the WHOLE psum bank, so only the first
            # pair sharing a bank may issue it (it zeroes the second
            # pair's region too); the second pair accumulates from zero.
            for lm in range(LM):
                for p in range(NPAIR):
                    reg = ctx_ps[b][p // 2][:, (p % 2) * CTXW:
                                            (p % 2 + 1) * CTXW]
                    nc.tensor.matmul(
                        reg,
                        E_t[lm][:, p * P:(p + 1) * P],
                        vT_t[lm][:, p * CTXW:(p + 1) * CTXW],
                        start=(i == 0 and lm == 0 and p % 2 == 0),
                        stop=(i == NCHUNK - 1 and lm == LM - 1),
                        skip_group_check=(p % 2 == 1))

    def finalize(b):
        # normalize ctx rows by the accumulated rowsum -> block-diag bf16
        ctxP[b] = []
        for p in range(NPAIR):
            acc = ctx_ps[b][p // 2]
            base = (p % 2) * CTXW
            r = rpool.tile([P, 1], F32, tag="recip", name="recip")
            nc.vector.reciprocal(r[0:DH, 0:1],
                                 acc[0:DH, base + DH:base + DH + 1])
            nc.vector.reciprocal(r[DH:P, 0:1],
                                 acc[DH:P, base + CTXW - 2:base + CTXW - 1])
            t = cpool.tile([P, P], BF16, tag="ctxP", name="ctxP")
            nc.vector.memset(t[:], 0.0)
            nc.vector.tensor_scalar_mul(t[0:DH, 0:DH],
                                        acc[0:DH, base:base + DH],
                                        r[0:DH, 0:1])
            nc.vector.tensor_scalar_mul(t[DH:P, DH:P],
                                        acc[DH:P, base + VW:base + VW + DH],
                                        r[DH:P, 0:1])
            ctxP[b].append(t)

    def build_m_and_pass2(b):
        # A_pair = ctxn_pair^T @ Wq_pair : [128 (h,e), 512 c]
        # PSUM->SBUF copies split across vector/scalar so neither engine's
        # queue lags the tensor engine.
        A_bf = []
        for p in range(NPAIR):
            ps = psmm.tile([P, DIM], F32, tag="mm", name="mm")
            nc.tensor.matmul(ps[:], ctxP[b][p][:],
                             wq_bf[:, p * DIM:(p + 1) * DIM],
                             start=True, stop=True)
            a = apool.tile([P, DIM], BF16, tag="A", name="A")
            if p % 2 == 0:
                nc.vector.tensor_copy(a[:], ps[:])
            else:
                nc.scalar.copy(a[:], ps[:])
            A_bf.append(a)
        # M^T[c, o] = sum_pairs A_pair[he, c]^T-contract WoutT_pair[he, o]
        Mt_bf = []
        for ct in range(KT):
            ps = psmm.tile([P, DIM], F32, tag="mm", name="mm")
            for p in range(NPAIR):
                nc.tensor.matmul(ps[:], A_bf[p][:, ct * P:(ct + 1) * P],
                                 wout_bf[:, p * DIM:(p + 1) * DIM],
                                 start=(p == 0), stop=(p == NPAIR - 1))
            m = mpool.tile([P, DIM], BF16, tag="Mt", name="Mt")
            if ct % 2 == 0:
                nc.vector.tensor_copy(m[:], ps[:])
            else:
                nc.scalar.copy(m[:], ps[:])
            Mt_bf.append(m)
        # Pass 2: y = M x + bias.  The bias-add drains PSUM; 4 adds/chunk
        # on vector alone (4.3us) would outpace tensor (3.4us/chunk), so
        # route one per chunk through scalar (activation Copy with bias).
        for i in range(NCHUNK):
            ls = slice(i * CHUNK, (i + 1) * CHUNK)
            yw = ypool.tile([P, MT * CHUNK], F32, tag="ysb", name="ysb")
            for m in range(MT):
                if m == MT - 1:
                    ps = psctx.tile([P, CHUNK], F32, tag="mm2", name="mm2")
                else:
                    ps = psmm.tile([P, CHUNK], F32, tag="mm", name="mm")
                for ct in range(KT):
                    nc.tensor.matmul(ps[:], Mt_bf[ct][:, m * P:(m + 1) * P],
                                     x_bf[b][ct][:, ls],
                                     start=(ct == 0), stop=(ct == KT - 1))
                yv = yw[:, m * CHUNK:(m + 1) * CHUNK]
                if m == MT - 1:
                    nc.scalar.add(yv, ps[:], bias_sb[:, m:m + 1])
                else:
                    nc.vector.tensor_scalar_add(yv, ps[:],
                                                bias_sb[:, m:m + 1])
                if b == BPC - 1 and i == NCHUNK - 1:
                    # final chunk: per-m DMAs so the kernel-end drain waits
                    # on 0.5 MB, not one 2 MB transfer
                    nc.sync.dma_start(y_out[b, m * P:(m + 1) * P, ls], yv)
            if not (b == BPC - 1 and i == NCHUNK - 1):
                nc.sync.dma_start(
                    y_out[b, :, ls].rearrange("(m p) l -> p m l", p=P),
                    yw[:].rearrange("p (m l) -> p m l", m=MT))

    # Issue order keeps the tensor queue dense: both k/v sweeps back-to-back
    # (finalize is vector-only and overlaps), then the M-build + y passes.
    pass1(0)
    load_late_weights()
    finalize(0)
    pass1(1)
    finalize(1)
    build_m_and_pass2(0)
    build_m_and_pass2(1)


def build_module():
    nc = bacc.Bacc("TRN2", target_bir_lowering=False, debug=False,
                   num_devices=NCORES)
    x_in = nc.dram_tensor("x", [BPC, DIM, L], F32R, kind="ExternalInput")
    wkvT_in = nc.dram_tensor("w_kvT", [DIM, 2 * HIDDEN], F32R,
                             kind="ExternalInput")
    wq_in = nc.dram_tensor("w_q", [HIDDEN, DIM], F32, kind="ExternalInput")
    woutT_in = nc.dram_tensor("w_outT", [HIDDEN, DIM], F32,
                              kind="ExternalInput")
    bias_in = nc.dram_tensor("bias", [P, MT], F32, kind="ExternalInput")
    y_out = nc.dram_tensor("y", [BPC, DIM, L], F32, kind="ExternalOutput")
    with tile.TileContext(nc) as tc:
        with ExitStack() as ctx:
            build_kernel(ctx, tc, x_in, wkvT_in, wq_in, woutT_in, bias_in,
                         y_out)
    nc.compile()
    return nc


def make_in_maps(x, w_qkv, w_out, b_out):
    x = np.ascontiguousarray(x, dtype=np.float32).reshape(B, DIM, L)
    w_qkv = np.asarray(w_qkv, dtype=np.float32)
    wkvT = np.ascontiguousarray(w_qkv.T[:, HIDDEN:3 * HIDDEN])
    wq = np.ascontiguousarray(w_qkv[0:HIDDEN, :])
    woutT = np.ascontiguousarray(np.asarray(w_out, dtype=np.float32).T)
    bias = np.ascontiguousarray(
        np.asarray(b_out, dtype=np.float32).reshape(MT, P).T)
    in_maps = []
    for c in range(NCORES):
        in_maps.append({
            "x": x[c * BPC:(c + 1) * BPC],
            "w_kvT": wkvT,
            "w_q": wq,
            "w_outT": woutT,
            "bias": bias,
        })
    return in_maps


_NC_CACHE = None


def kernel(x, w_qkv, w_out, b_out, *, trace=False, trace_kwargs=None):
    """Full inputs in, full output out. Shards batch across 8 NeuronCores."""
    global _NC_CACHE
    from concourse.bass_utils import run_bass_kernel_spmd

    if _NC_CACHE is None:
        _NC_CACHE = build_module()
    nc = _NC_CACHE

    in_maps = make_in_maps(x, w_qkv, w_out, b_out)
    kw = dict(trace_kwargs or {})
    res = run_bass_kernel_spmd(nc, in_maps, list(range(NCORES)),
                               trace=trace, **kw)
    y = np.empty((B, DIM, HGT, WID), dtype=np.float32)
    for c in range(NCORES):
        y[c * BPC:(c + 1) * BPC] = res.results[c]["y"].reshape(
            BPC, DIM, HGT, WID)
    kernel.last_results = res
    return y


# revision 34
# speedup vs baseline: 1.0316x; 1.0316x over previous
# Trainium2 Bass kernel for LinearAttention (nn_LinearAttention_87686052315975).
#
# Reference computation (per batch element b of 16):
#   xf = x[b].reshape(512, 4096)                      # [c, l]
#   qkv = w_qkv @ xf; q, k, v split into 8 heads x 64 dims
#   k = softmax(k, axis=l)
#   context_h = k_h @ v_h^T                           # [64, 64]
#   out_h = context_h^T @ q_h                          # [64, l]
#   y = w_out @ concat(out_h) + b_out                 # [512, l]
#
# Key restructure vs a direct mapping: since context_h is tiny, fold it into
# the weights.  y = sum_h Wout_h ctxn_h^T Wq_h x = M x with M [512, 512]
# depending only on ctx (data-dependent) and the fixed weights.  This removes
# the q projection (q never materialized) and the per-l attention pass;
# after the k/v sweep we build M (~10k PE cycles) and do one plain matmul
# y = M x + bias.
#
# Per-batch structure (2 batches per core, data-parallel over 8 cores):
#   Pass 1 (l chunked by 512):  kT/vT computed transposed (l on partitions)
#     so the context contraction over l maps onto the PE K dim; E = exp(kT)
#     cast to bf16; vT cast to bf16 with a ones column per head appended so
#     the context matmul also accumulates rowsum(E) (softmax denominator).
#     ctx accumulates in PSUM across all 32 l-subtiles (2 head-pairs per
#     bank, block-diagonal packing).  bf16 runs the N=132 context matmuls at
#     1 cycle/row (fp32r would pay 4x at N<256).  x is also cast to a
#     resident bf16 copy for pass 2.
#   Finalize:  ctxn = ctx * (1/rowsum) into block-diag bf16 tiles.
#   Build M:   A_pair = ctxn_pair^T-contract-Wq_pair  [128, 512]
#              M^T[c, o] = sum_pairs A_pair^T-contract-WoutT_pair
#   Pass 2:    y = (M^T)^T-contract-x_bf16 + bias; DMA out.
#
# Big fp32 matmuls (k/v projection) run as float32r (1 cycle/row at N>=512).
# Everything downstream of exp runs bf16 (inputs only; PSUM accumulation is
# fp32) — well inside the 2e-2 tolerance.

import numpy as np
from contextlib import ExitStack

import concourse.bass as bass
import concourse.bacc as bacc
import concourse.mybir as mybir
import concourse.tile as tile

# ---- problem constants (hardcoded per contract) ----
B, DIM, HGT, WID = 16, 512, 64, 64
L = HGT * WID            # 4096
HEADS, DH = 8, 64
HIDDEN = HEADS * DH      # 512
NCORES = 8
BPC = B // NCORES        # 2 batches per core
P = 128
CHUNK = 512
NCHUNK = L // CHUNK      # 8
KT = DIM // P            # 4 contraction tiles over channels
MT = DIM // P            # 4 output row tiles
LM = CHUNK // P          # 4 l-subtiles per chunk
NPAIR = HEADS // 2       # 4 head pairs
VW = DH + 2              # per-head vT width: 64 v cols + 2 ones cols (even N)
CTXW = 2 * VW            # 132: one pair's context block width

F32 = mybir.dt.float32
F32R = mybir.dt.float32r
BF16 = mybir.dt.bfloat16


def _f32(ap):
    return ap.bitcast(F32)


def build_kernel(ctx: ExitStack, tc: "tile.TileContext", x_in, wkvT_in, wq_in,
                 woutT_in, bias_in, y_out):
    nc = tc.nc

    wpool = ctx.enter_context(tc.tile_pool(name="weights", bufs=1))
    xpool = ctx.enter_context(tc.tile_pool(name="xc", bufs=8))
    xbpool = ctx.enter_context(tc.tile_pool(name="xbf", bufs=8))
    epool = ctx.enter_context(tc.tile_pool(name="ev", bufs=8))
    cpool = ctx.enter_context(tc.tile_pool(name="ctxp", bufs=8))
    apool = ctx.enter_context(tc.tile_pool(name="absf", bufs=4))
    mpool = ctx.enter_context(tc.tile_pool(name="mtbf", bufs=8))
    rpool = ctx.enter_context(tc.tile_pool(name="recip", bufs=8))
    ypool = ctx.enter_context(tc.tile_pool(name="ysb", bufs=5))
    psmm = ctx.enter_context(tc.tile_pool(name="psmm", bufs=4, space="PSUM"))
    # "ctx" needs only 2 bufs (finalize(b) drains before pass1(b+1)
    # allocates); the freed banks serve as extra pass-2 accumulators so the
    # tensor engine isn't gated on the bias-add drain at chunk boundaries.
    psctx = ctx.enter_context(tc.tile_pool(name="psctx", bufs=2, space="PSUM"))

    # ---- load k/v weights + bias up front; wq/wout deferred (only needed
    # by build-M, which runs after both k/v sweeps) so the first x chunk's
    # DMAs aren't queued behind weight traffic.
    wkv_sb = [wpool.tile([P, 2 * HIDDEN], F32R, tag=f"wkv{k}", name=f"wkv{k}")
              for k in range(KT)]
    bias_sb = wpool.tile([P, MT], F32, tag="bias", name="bias")
    nc.sync.dma_start(bias_sb[:], bias_in[:])
    wq_bf = wpool.tile([P, KT * DIM], BF16, tag="wq", name="wq")
    wout_bf = wpool.tile([P, KT * DIM], BF16, tag="wout", name="wout")

    def load_late_weights():
        # staged fp32 loads + on-chip bf16 casts (gpsimd SWDGE could cast in
        # flight, but its teardown costs ~12us of epilogue on the trace)
        sq = xpool.tile([P, KT * DIM], F32, tag="stg", name="wq_stage",
                        bufs=2)
        nc.sync.dma_start(
            sq[:].rearrange("p (k c) -> p k c", k=KT),
            wq_in[:, :].rearrange("(k p) c -> p k c", p=P))
        nc.vector.tensor_copy(wq_bf[:], sq[:])
        so = xpool.tile([P, KT * DIM], F32, tag="stg", name="wout_stage",
                        bufs=2)
        nc.sync.dma_start(
            so[:].rearrange("p (k c) -> p k c", k=KT),
            woutT_in[:, :].rearrange("(k p) c -> p k c", p=P))
        nc.scalar.copy(wout_bf[:], so[:])

    x_bf = {}      # batch -> 4 resident bf16 tiles [128, 4096]
    ctxP = {}      # batch -> 4 block-diag bf16 [128, 128] normalized ctx
    ctx_ps = {}    # batch -> 2 PSUM tiles [128, 264] (2 pairs each)

    def pass1(b):
        x_bf[b] = [xbpool.tile([P, L], BF16, tag="xbf", name=f"xbf{b}_{k}")
                   for k in range(KT)]
        ctx_ps[b] = [psctx.tile([P, 2 * CTXW], F32, tag="ctx", name="ctx")
                     for _ in range(2)]
        DBL = 2 * CHUNK
        for i in range(NCHUNK):
            ls = slice(i * CHUNK, (i + 1) * CHUNK)
            # x loads cover chunk PAIRS so dram row segments are 4 KB (not
            # 2 KB), doubling throughput in the descriptor-latency-bound
            # ramp-up window; each pair is TWO 1 MB dmas (2 k-tiles each) so
            # completion granularity matches the per-chunk scheme.  The very
            # first chunk stays split per k-tile, k-projection weights first,
            # so the first matmul only waits on ~384 KB.
            if i % 2 == 0:
                xcw = xpool.tile([P, KT * DBL], F32R, tag="xc", name="xcw",
                                 bufs=2)
                pass1.xcw = xcw
                if b == 0 and i == 0:
                    for k in range(KT):
                        nc.sync.dma_start(
                            xcw[:, k * DBL:k * DBL + CHUNK // 2],
                            x_in[b, k * P:(k + 1) * P, 0:CHUNK // 2])
                        nc.sync.dma_start(wkv_sb[k][:, 0:HIDDEN],
                                          wkvT_in[k * P:(k + 1) * P,
                                                  0:HIDDEN])
                        nc.sync.dma_start(wkv_sb[k][:, HIDDEN:2 * HIDDEN],
                                          wkvT_in[k * P:(k + 1) * P,
                                                  HIDDEN:2 * HIDDEN])
                    for k in range(KT):
                        nc.sync.dma_start(
                            xcw[:, k * DBL + CHUNK // 2:k * DBL + CHUNK],
                            x_in[b, k * P:(k + 1) * P, CHUNK // 2:CHUNK])
                    nc.sync.dma_start(
                        xcw[:].rearrange("p (k l) -> p k l", k=KT)[
                            :, :, CHUNK:DBL],
                        x_in[b, :, CHUNK:DBL].rearrange(
                            "(k p) l -> p k l", p=P))
                else:
                    for g in range(2):
                        nc.sync.dma_start(
                            xcw[:, 2 * g * DBL:(2 * g + 2) * DBL].rearrange(
                                "p (k l) -> p k l", k=2),
                            x_in[b, 2 * g * P:(2 * g + 2) * P,
                                 i * CHUNK:(i + 2) * CHUNK].rearrange(
                                "(k p) l -> p k l", p=P))
            else:
                xcw = pass1.xcw
            off = (i % 2) * CHUNK
            xc = [xcw[:, k * DBL + off:k * DBL + off + CHUNK]
                  for k in range(KT)]
            for k in range(KT):
                nc.scalar.copy(x_bf[b][k][:, ls], _f32(xc[k]))

            E_t, vT_t = [], []
            for lm in range(LM):
                lms = slice(lm * P, (lm + 1) * P)
                # kT: [128 l, 512 (h,d)] -> E = exp
                ps = psmm.tile([P, CHUNK], F32, tag="mm", name="mm")
                for k in range(KT):
                    nc.tensor.matmul(ps[:], xc[k][:, lms],
                                     wkv_sb[k][:, 0:HIDDEN],
                                     start=(k == 0), stop=(k == KT - 1))
                e = epool.tile([P, CHUNK], BF16, tag="E", name="E")
                nc.scalar.activation(e[:], ps[:],
                                     mybir.ActivationFunctionType.Exp)
                E_t.append(e)
                # vT: [128 l, 512 (h,e)] -> bf16 with ones cols per head
                ps = psmm.tile([P, CHUNK], F32, tag="mm", name="mm")
                for k in range(KT):
                    nc.tensor.matmul(ps[:], xc[k][:, lms],
                                     wkv_sb[k][:, HIDDEN:2 * HIDDEN],
                                     start=(k == 0), stop=(k == KT - 1))
                v = epool.tile([P, HEADS * VW], BF16, tag="vT", name="vT")
                v_view = v[:].rearrange("p (h e) -> p h e", e=VW)
                nc.vector.tensor_copy(
                    v_view[:, :, 0:DH],
                    ps[:].rearrange("p (h e) -> p h e", e=DH))
                nc.vector.memset(v_view[:, :, DH:VW], 1.0)
                vT_t.append(v)

            # context accumulation into persistent PSUM, one matmul per
            # head pair (block-diag packing; off-diag blocks never read).
            # start=True resets the WHOLE psum bank, so only the first
            # pair sharing a bank may issue it (it zeroes the second
            # pair's region too); the second pair accumulates from zero.
            for lm in range(LM):
                for p in range(NPAIR):
                    reg = ctx_ps[b][p // 2][:, (p % 2) * CTXW:
                                            (p % 2 + 1) * CTXW]
                    nc.tensor.matmul(
                        reg,
                        E_t[lm][:, p * P:(p + 1) * P],
                        vT_t[lm][:, p * CTXW:(p + 1) * CTXW],
                        start=(i == 0 and lm == 0 and p % 2 == 0),
                        stop=(i == NCHUNK - 1 and lm == LM - 1),
                        skip_group_check=(p % 2 == 1))

    def finalize(b):
        # normalize ctx rows by the accumulated rowsum -> block-diag bf16
        ctxP[b] = []
        for p in range(NPAIR):
            acc = ctx_ps[b][p // 2]
            base = (p % 2) * CTXW
            r = rpool.tile([P, 1], F32, tag="recip", name="recip")
            nc.vector.reciprocal(r[0:DH, 0:1],
                                 acc[0:DH, base + DH:base + DH + 1])
            nc.vector.reciprocal(r[DH:P, 0:1],
                                 acc[DH:P, base + CTXW - 2:base + CTXW - 1])
            t = cpool.tile([P, P], BF16, tag="ctxP", name="ctxP")
            nc.vector.memset(t[:], 0.0)
            nc.vector.tensor_scalar_mul(t[0:DH, 0:DH],
                                        acc[0:DH, base:base + DH],
                                        r[0:DH, 0:1])
            nc.vector.tensor_scalar_mul(t[DH:P, DH:P],
                                        acc[DH:P, base + VW:base + VW + DH],
                                        r[DH:P, 0:1])
            ctxP[b].append(t)

    def build_m_and_pass2(b):
        # A_pair = ctxn_pair^T @ Wq_pair : [128 (h,e), 512 c]
        # PSUM->SBUF copies split across vector/scalar so neither engine's
        # queue lags the tensor engine.
        A_bf = []
        for p in range(NPAIR):
            ps = psmm.tile([P, DIM], F32, tag="mm", name="mm")
            nc.tensor.matmul(ps[:], ctxP[b][p][:],
                             wq_bf[:, p * DIM:(p + 1) * DIM],
                             start=True, stop=True)
            a = apool.tile([P, DIM], BF16, tag="A", name="A")
            if p % 2 == 0:
                nc.vector.tensor_copy(a[:], ps[:])
            else:
                nc.scalar.copy(a[:], ps[:])
            A_bf.append(a)
        # M^T[c, o] = sum_pairs A_pair[he, c]^T-contract WoutT_pair[he, o]
        Mt_bf = []
        for ct in range(KT):
            ps = psmm.tile([P, DIM], F32, tag="mm", name="mm")
            for p in range(NPAIR):
                nc.tensor.matmul(ps[:], A_bf[p][:, ct * P:(ct + 1) * P],
                                 wout_bf[:, p * DIM:(p + 1) * DIM],
                                 start=(p == 0), stop=(p == NPAIR - 1))
            m = mpool.tile([P, DIM], BF16, tag="Mt", name="Mt")
            if ct % 2 == 0:
                nc.vector.tensor_copy(m[:], ps[:])
            else:
                nc.scalar.copy(m[:], ps[:])
            Mt_bf.append(m)
        # Pass 2: y = M x + bias.  The bias-add drains PSUM; 4 adds/chunk
        # on vector alone (4.3us) would outpace tensor (3.4us/chunk), so
        # route one per chunk through scalar (activation Copy with bias).
        for i in range(NCHUNK):
            ls = slice(i * CHUNK, (i + 1) * CHUNK)
            yw = ypool.tile([P, MT * CHUNK], F32, tag="ysb", name="ysb")
            for m in range(MT):
                if m == MT - 1:
                    ps = psctx.tile([P, CHUNK], F32, tag="mm2", name="mm2")
                else:
                    ps = psmm.tile([P, CHUNK], F32, tag="mm", name="mm")
                for ct in range(KT):
                    nc.tensor.matmul(ps[:], Mt_bf[ct][:, m * P:(m + 1) * P],
                                     x_bf[b][ct][:, ls],
                                     start=(ct == 0), stop=(ct == KT - 1))
                yv = yw[:, m * CHUNK:(m + 1) * CHUNK]
                if m == MT - 1:
                    nc.scalar.add(yv, ps[:], bias_sb[:, m:m + 1])
                else:
                    nc.vector.tensor_scalar_add(yv, ps[:],
                                                bias_sb[:, m:m + 1])
                if b == BPC - 1 and i == NCHUNK - 1:
                    # final chunk: per-m DMAs so the kernel-end drain waits
                    # on 0.5 MB, not one 2 MB transfer
                    nc.sync.dma_start(y_out[b, m * P:(m + 1) * P, ls], yv)
            if not (b == BPC - 1 and i == NCHUNK - 1):
                nc.sync.dma_start(
                    y_out[b, :, ls].rearrange("(m p) l -> p m l", p=P),
                    yw[:].rearrange("p (m l) -> p m l", m=MT))

    # Issue order keeps the tensor queue dense: both k/v sweeps back-to-back
    # (finalize is vector-only and overlaps), then the M-build + y passes.
    pass1(0)
    load_late_weights()
    finalize(0)
    pass1(1)
    finalize(1)
    build_m_and_pass2(0)
    build_m_and_pass2(1)


def build_module():
    nc = bacc.Bacc("TRN2", target_bir_lowering=False, debug=False,
                   num_devices=NCORES)
    x_in = nc.dram_tensor("x", [BPC, DIM, L], F32R, kind="ExternalInput")
    wkvT_in = nc.dram_tensor("w_kvT", [DIM, 2 * HIDDEN], F32R,
                             kind="ExternalInput")
    wq_in = nc.dram_tensor("w_q", [HIDDEN, DIM], F32, kind="ExternalInput")
    woutT_in = nc.dram_tensor("w_outT", [HIDDEN, DIM], F32,
                              kind="ExternalInput")
    bias_in = nc.dram_tensor("bias", [P, MT], F32, kind="ExternalInput")
    y_out = nc.dram_tensor("y", [BPC, DIM, L], F32, kind="ExternalOutput")
    with tile.TileContext(nc) as tc:
        with ExitStack() as ctx:
            build_kernel(ctx, tc, x_in, wkvT_in, wq_in, woutT_in, bias_in,
                         y_out)
    nc.compile()
    return nc


def make_in_maps(x, w_qkv, w_out, b_out):
    x = np.ascontiguousarray(x, dtype=np.float32).reshape(B, DIM, L)
    w_qkv = np.asarray(w_qkv, dtype=np.float32)
    wkvT = np.ascontiguousarray(w_qkv.T[:, HIDDEN:3 * HIDDEN])
    wq = np.ascontiguousarray(w_qkv[0:HIDDEN, :])
    woutT = np.ascontiguousarray(np.asarray(w_out, dtype=np.float32).T)
    bias = np.ascontiguousarray(
        np.asarray(b_out, dtype=np.float32).reshape(MT, P).T)
    in_maps = []
    for c in range(NCORES):
        in_maps.append({
            "x": x[c * BPC:(c + 1) * BPC],
            "w_kvT": wkvT,
            "w_q": wq,
            "w_outT": woutT,
            "bias": bias,
        })
    return in_maps


_NC_CACHE = None


def kernel(x, w_qkv, w_out, b_out, *, trace=False, trace_kwargs=None):
    """Full inputs in, full output out. Shards batch across 8 NeuronCores."""
    global _NC_CACHE
    from concourse.bass_utils import run_bass_kernel_spmd

    if _NC_CACHE is None:
        _NC_CACHE = build_module()
    nc = _NC_CACHE

    in_maps = make_in_maps(x, w_qkv, w_out, b_out)
    kw = dict(trace_kwargs or {})
    res = run_bass_kernel_spmd(nc, in_maps, list(range(NCORES)),
                               trace=trace, **kw)
    y = np.empty((B, DIM, HGT, WID), dtype=np.float32)
    for c in range(NCORES):
        y[c * BPC:(c + 1) * BPC] = res.results[c]["y"].reshape(
            BPC, DIM, HGT, WID)
    kernel.last_results = res
    return y
